# revision 2
# baseline (speedup 1.0000x reference)
"""CapsuleLayer kernel for Trainium2 (8 NeuronCores, data-parallel over batch).

Math: the reference's dynamic-routing loop is degenerate — `delta` is summed
over the capsule axis and broadcast back, so the logits stay constant across
capsules and softmax stays uniform (1/16) for all 3 iterations. The module
therefore reduces exactly to

    t   = (conv2d(x, sum_c W[c]) + sum_c b[c]) / 16      # 16-out-channel conv
    out = sign(t) * t^2 / (1 + t^2)                      # scalar squash

The capsule sum is folded into the conv weights on the host (conv is linear in
the weights), leaving a [O=16, I=64, 3, 3] VALID conv + pointwise epilogue.

Device strategy per core (8 images per core, one image PAIR per wave):
  - x for an image pair lives in SBUF as [128, 66, 66] (partitions = parity*64
    + in_channel), loaded with one contiguous ~1.1 MB DMA.
  - The conv runs on the TensorEngine as 9-tap accumulating matmuls packed
    8-wide into the 128x128 array with tile_position (2 row groups = image
    parity rg, 4 col groups j). Each 32-wide column group carries TWO h-tiles:
    per (tap, rg, j) we issue two matmuls whose [64,32] stationary blocks are
    zero-masked on opposite 16-column halves (cols 16*half..16*half+16 hold
    the tap weights). Zeros accumulate harmlessly into the other half's PSUM
    partitions, so ALL 128 PSUM partitions hold valid conv outputs:
      partition = 32*j + 16*half + o   (h-tile ht = 2*j+half, out channel o)
      free      = 512*rg + 64*hr + w   (hr = row within h-tile, w = col)
    PSUM tile per pair: [128, 1024] f32 = 2 banks. Same PE streaming cycles
    as the half-empty layout, but the epilogue free-dim halves and the output
    DMA is fully dense.
  - Epilogue (exact for any bias b):
      u = Square(ps + b)      [ScalarE]
      s = Sign(ps + b)        [ScalarE, bf16]
      w1 = u + 1              [DVE 2x_2P]
      r = 1/w1                [DVE reciprocal_approx_fast]
      c = 1 - r = u/(1+u)     [DVE tensor_scalar affine, bf16]
      f = s * c               [DVE tensor_tensor, bf16]
  - f [128, 1024] bf16 DMAs densely into out [4, 128, 1024]; unshuffled on
    the host.
"""

import numpy as np

N_CORES = 8
B_PER_CORE = 8  # 64 images / 8 cores


def _build_nc(
    repeat=1,
    loop_repeat=1,
    conv_bf16=False,
    parts=None,
    x_bufs=2,
    ps_bufs=3,
):
    # parts: subset of {"in", "mm", "epi", "out"} for bench attribution;
    # None = all.
    if parts is None:
        parts = {"in", "mm", "epi", "out"}
    import contextlib

    import concourse.bacc as bacc
    import concourse.mybir as mybir
    import concourse.tile as tile

    f32 = mybir.dt.float32
    cdt = mybir.dt.bfloat16 if conv_bf16 else f32
    # Bacc (not raw Bass): its finalize() runs move_matmul_waits_to_ldweights
    # + generate_event_semaphores, required for TRN2's 1-wait-per-instruction
    # limit (our first matmuls collect several Tile sem waits).
    nc = bacc.Bacc(None, target_bir_lowering=False, debug=False)

    x_d = nc.dram_tensor("x", [512, 66, 66], cdt, kind="ExternalInput")
    w_d = nc.dram_tensor("w", [128, 576], cdt, kind="ExternalInput")
    bv_d = nc.dram_tensor("bvec", [128, 1], f32, kind="ExternalInput")
    # Raw per-pair dump [pair, partition, 512*rg + 64*hr + w]; unshuffled on
    # the host.
    out_d = nc.dram_tensor("out", [4, 128, 1024], cdt, kind="ExternalOutput")

    with tile.TileContext(nc) as tc:
        with (
            tc.tile_pool(name="const", bufs=1) as cp,
            tc.tile_pool(name="xp", bufs=x_bufs) as xp,
            tc.tile_pool(name="psp", bufs=ps_bufs, space="PSUM") as psp,
            tc.tile_pool(name="wk", bufs=2) as wk,
        ):
            w_t = cp.tile([128, 576], cdt)
            nc.sync.dma_start(out=w_t[:, :], in_=w_d[:, :])
            b_t = cp.tile([128, 1], f32)
            nc.sync.dma_start(out=b_t[:, :], in_=bv_d[:, :])

            if loop_repeat > 1:  # bench only: HW loop repeating the body
                loop_cm = tc.For_i(
                    0,
                    loop_repeat,
                    1,
                    hint_engines=(
                        mybir.EngineType.PE,
                        mybir.EngineType.Activation,
                        mybir.EngineType.DVE,
                        mybir.EngineType.SP,
                    ),
                )
            else:
                loop_cm = contextlib.nullcontext()
            with loop_cm:
                for p4 in range(4 * repeat):
                    p = p4 % 4
                    x_t = xp.tile([128, 66, 66], cdt, tag="x")
                    if "in" in parts:
                        nc.sync.dma_start(
                            out=x_t[:, :, :], in_=x_d[128 * p : 128 * (p + 1), :, :]
                        )
                    ps = psp.tile([128, 1024], f32, tag="ps")

                    if "mm" in parts:
                        for t in range(9):
                            kh, kw = divmod(t, 3)
                            for half in range(2):
                                for rg in range(2):
                                    for j in range(4):
                                        h0 = (2 * j + half) * 8
                                        nc.tensor.matmul(
                                            ps[
                                                32 * j : 32 * j + 32,
                                                512 * rg : 512 * rg + 512,
                                            ],
                                            w_t[
                                                64 * rg : 64 * rg + 64,
                                                32 * (2 * t + half) : 32 * (2 * t + half)
                                                + 32,
                                            ],
                                            x_t[
                                                64 * rg : 64 * rg + 64,
                                                h0 + kh : h0 + kh + 8,
                                                kw : kw + 64,
                                            ],
                                            start=(t == 0 and half == 0),
                                            stop=(t == 8 and half == 1),
                                            tile_position=(64 * rg, 32 * j),
                                            skip_group_check=True,
                                        )

                    if "epi" in parts:
                        u = wk.tile([128, 1024], f32, tag="u")
                        s = wk.tile([128, 1024], cdt, tag="s")
                        w1 = wk.tile([128, 1024], f32, tag="w1")
                        r = wk.tile([128, 1024], f32, tag="r")
                        c = wk.tile([128, 1024], cdt, tag="c")
                        f = wk.tile([128, 1024], cdt, tag="f")
                        nc.scalar.activation(
                            u[:, :], ps[:, :], mybir.ActivationFunctionType.Square,
                            bias=b_t[:, 0:1],
                        )
                        nc.scalar.activation(
                            s[:, :], ps[:, :], mybir.ActivationFunctionType.Sign,
                            bias=b_t[:, 0:1],
                        )
                        nc.vector.tensor_scalar_add(w1[:, :], u[:, :], 1.0)
                        nc.vector.reciprocal_approx_fast(r[:, :], w1[:, :])
                        # c = 1 - r = t^2/(1+t^2)
                        nc.vector.tensor_scalar(
                            c[:, :], r[:, :], -1.0, 1.0,
                            mybir.AluOpType.mult, mybir.AluOpType.add,
                        )
                        nc.vector.tensor_mul(f[:, :], s[:, :], c[:, :])
                        if "out" in parts:
                            nc.sync.dma_start(out=out_d[p, :, :], in_=f[:, :])
    # Run the Bacc pass pipeline (wait splitting, reg alloc, ...) now; the
    # axon/pjrt execute path binds the primitive without finalizing.
    nc.finalize()
    return nc


def _np_bf16(a):
    import ml_dtypes

    return np.ascontiguousarray(a.astype(ml_dtypes.bfloat16))


def _prep_weights(W, b):
    """[16,16,64,3,3] capsule weights -> [128, 576] lhsT blocks (pre-summed
    over capsules, /16 for the uniform routing probs, duplicated into both
    partition halves; per (tap, half) a [64,32] block zero-masked outside
    cols 16*half..16*half+16).  Bias -> [128, 1] per-partition vector."""
    Wsum = np.asarray(W, dtype=np.float32).sum(axis=0) / 16.0  # [16, 64, 3, 3]
    w_arr = np.zeros((128, 576), np.float32)
    for t in range(9):
        kh, kw = divmod(t, 3)
        blk = np.ascontiguousarray(Wsum[:, :, kh, kw].T)  # [64 in, 16 out]
        for half in range(2):
            c0 = 32 * (2 * t + half) + 16 * half
            w_arr[0:64, c0 : c0 + 16] = blk
            w_arr[64:128, c0 : c0 + 16] = blk
    bsum = np.asarray(b, dtype=np.float32).sum(axis=0) / 16.0  # [16]
    bvec = np.zeros((128, 1), np.float32)
    for g in range(8):
        bvec[16 * g : 16 * g + 16, 0] = bsum
    return w_arr, bvec


def make_in_maps(x, W, b, conv_bf16=False):
    x = np.ascontiguousarray(np.asarray(x, dtype=np.float32))
    w_arr, bvec = _prep_weights(W, b)
    if conv_bf16:
        x = _np_bf16(x)
        w_arr = _np_bf16(w_arr)
    return [
        {
            "x": np.ascontiguousarray(
                x[c * B_PER_CORE : (c + 1) * B_PER_CORE].reshape(512, 66, 66)
            ),
            "w": w_arr,
            "bvec": bvec,
        }
        for c in range(N_CORES)
    ]


def gather_out(per_core_outs):
    """Unshuffle raw [4, 128, 1024] per-core dumps into [64, 65536, 1] f32.

    partition = 32*j + 16*half + o; free = 512*rg + 64*hr + w;
    out[b=2p+rg, o*4096 + (2j+half)*512 + 64*hr + w]."""
    full = np.empty((64, 65536), np.float32)
    for c, raw in enumerate(per_core_outs):
        r = np.asarray(raw, dtype=np.float32).reshape(4, 4, 2, 16, 2, 8, 64)
        # axes: [p, j, half, o, rg, hr, w] -> [p, rg, o, j, half, hr, w]
        v = r.transpose(0, 4, 3, 1, 2, 5, 6)
        full[c * 8 : (c + 1) * 8] = v.reshape(8, 65536)
    return full.reshape(64, 65536, 1)


def kernel(x, W, b):
    from concourse.bass_utils import run_bass_kernel_spmd

    nc = _build_nc(conv_bf16=True)
    in_maps = make_in_maps(x, W, b, conv_bf16=True)
    res = run_bass_kernel_spmd(nc, in_maps, list(range(N_CORES)))
    return gather_out([res.results[c]["out"] for c in range(N_CORES)])
